# revision 17
# baseline (speedup 1.0000x reference)
"""Trainium2 Bass kernel for nn_LocallyDense (grouped gather + per-group Dense
+ LeakyReLU + BatchNorm inference).

Sharding: expert-parallel over output half-units. A (group, o-half) pair is a
unit; there are 41*2 = 82 units. Each core runs 11 units: 5 full groups
(both halves) + 1 single half. 8*11 = 88 units cover 82 with 6 padded
(duplicate) units whose output is discarded.

The gather runs on the HOST during sharding prep: each core receives its
groups' x-columns already gathered AND packed into tile layout
([128, cols]: partition = k%128, free = (k-block, batch)), so the device
program is a pure streaming grouped GEMM with a TRANSPOSED output
(psum[o, b] via lhsT=W): per-output-channel constants (bias, BN scale/shift)
are per-partition scalars riding the ACT/DVE instructions.

Per unit (z = x@W + b, p = psum = x@W):
  rt = Relu((1-a)*p + (1-a)*b)        # ACT, per-partition bias AP
  ot = a*p + rt = leaky(z) - a*b      # DVE scalar_tensor_tensor
  y  = ot*inv + (c + a*b*inv)         # DVE tensor_scalar -> bf16
where inv = gamma/sqrt(var+eps), c = beta - mean*inv (host-computed).

Engine/queue layout (engines are in-order; loads are emitted before compute
so prefetch depth is limited only by buffer count — all input tiles are
fully resident in SBUF):
  sync:   wt0+xg0 split in half-chunks (fast pipeline start), xg1..xg4
  scalar: wt1..wt4, wt5h, xg5
  gpsimd: cst, output stores
"""

import numpy as np
import ml_dtypes

B, D_IN, N_GROUPS, G, D_OUT = 256, 65536, 41, 1536, 256
BN_EPS = 1e-3
ALPHA = 0.3
N_CORES = 8
NF = 5                # full groups per core
NU = 11               # half-units per core (5*2 + 1)
NXS = 6               # x slots (5 full groups + the half unit's group)
KT = G // 128         # 12 K-tiles per group
KB2 = KT * B          # x cols per group slot (3072)
WF = KT * D_OUT       # w cols per full group (3072)
WH = KT * 128         # w cols for the half slot (1536)
XCOLS = NXS * KB2
WCOLS = NF * WF       # full-group W table; half chunk is separate tensor
NCOL = 3 * NU         # const table: biasS | c2 | inv per unit

USE_BF16 = True       # x/W feed the PE in bf16 (fp32 accumulate in PSUM)
TRACE = False         # set by test.py for profiling runs
TRACE_KW = {}
REPEAT = 1

_prog_cache = {}


def _np_dtx():
    return ml_dtypes.bfloat16 if USE_BF16 else np.float32


def _build_program(use_bf16: bool):
    import concourse.bacc as bacc
    import concourse.mybir as mybir
    import concourse.tile as tile

    f32 = mybir.dt.float32
    dt_x = mybir.dt.bfloat16 if use_bf16 else mybir.dt.float32

    nc = bacc.Bacc("TRN2", target_bir_lowering=False, debug=False,
                   num_devices=N_CORES)
    xg = nc.dram_tensor("xg", [128, XCOLS], dt_x, kind="ExternalInput")
    wt = nc.dram_tensor("wt", [128, WCOLS], dt_x, kind="ExternalInput")
    wh = nc.dram_tensor("wh", [128, WH], dt_x, kind="ExternalInput")
    cst = nc.dram_tensor("cst", [128, NCOL], f32, kind="ExternalInput")
    out = nc.dram_tensor("out", [NU * 128, B], dt_x, kind="ExternalOutput")

    with tile.TileContext(nc) as tc:
        with tc.tile_pool(name="const", bufs=1) as cpool, \
             tc.tile_pool(name="xp", bufs=NXS) as xpool, \
             tc.tile_pool(name="wp", bufs=NF) as wpool, \
             tc.tile_pool(name="ep", bufs=4) as epool, \
             tc.tile_pool(name="ot", bufs=6) as opool, \
             tc.tile_pool(name="ps", bufs=3, space="PSUM") as ppool:

            # ---- load stream (emitted before compute; in-order queues) ----
            # cst first on scalar: tiny, warms the scalar HWDGE ring
            cst_t = cpool.tile([128, NCOL], f32)
            nc.scalar.dma_start(out=cst_t[:], in_=cst[:, :])

            xts = [None] * NXS
            wts = [None] * NF
            # group 0 in third-chunks on sync for a fast pipeline start
            wt0 = wpool.tile([128, WF], dt_x, tag="wg", name="wt_0")
            xg0 = xpool.tile([128, KB2], dt_x, tag="xg", name="xt_0")
            for ci in range(3):
                wl, wr = ci * WF // 3, (ci + 1) * WF // 3
                xl, xr = ci * KB2 // 3, (ci + 1) * KB2 // 3
                nc.sync.dma_start(out=wt0[:, wl:wr], in_=wt[:, wl:wr])
                nc.sync.dma_start(out=xg0[:, xl:xr], in_=xg[:, xl:xr])
            xts[0], wts[0] = xg0, wt0
            # group 1 early on scalar (right after cst)
            wts[1] = wpool.tile([128, WF], dt_x, tag="wg", name="wt_1")
            nc.scalar.dma_start(out=wts[1][:], in_=wt[:, WF:2 * WF])
            xts[1] = xpool.tile([128, KB2], dt_x, tag="xg", name="xt_1")
            nc.scalar.dma_start(out=xts[1][:], in_=xg[:, KB2:2 * KB2])
            # remaining x on sync, remaining W on scalar
            for g in range(2, NF):
                xts[g] = xpool.tile([128, KB2], dt_x, tag="xg",
                                    name=f"xt_{g}")
                nc.sync.dma_start(
                    out=xts[g][:], in_=xg[:, g * KB2:(g + 1) * KB2])
            xts[NF] = xpool.tile([128, KB2], dt_x, tag="xg", name="xt_5")
            nc.sync.dma_start(out=xts[NF][:], in_=xg[:, NF * KB2:NXS * KB2])
            for g in range(2, NF):
                wts[g] = wpool.tile([128, WF], dt_x, tag="wg",
                                    name=f"wt_{g}")
                nc.scalar.dma_start(
                    out=wts[g][:], in_=wt[:, g * WF:(g + 1) * WF])
            wt5h = cpool.tile([128, WH], dt_x, name="wt5h")
            nc.scalar.dma_start(out=wt5h[:], in_=wh[:, :])

            # ---- compute stream ----
            for u_rep in range(REPEAT * NU):
                u = u_rep % NU
                if u < 2 * NF:
                    g, h = u // 2, u % 2
                    xt_g, wt_g = xts[g], wts[g]

                    def lhsT(blk, _w=wt_g, _h=h):
                        return _w[:, blk * D_OUT + _h * 128:
                                  blk * D_OUT + (_h + 1) * 128]
                else:
                    xt_g = xts[NF]

                    def lhsT(blk, _w=wt5h):
                        return _w[:, blk * 128:(blk + 1) * 128]

                ps = ppool.tile([128, B], f32, tag=f"ps{u % 2}",
                                name=f"ps_{u_rep}")
                for blk in range(KT):
                    nc.tensor.matmul(
                        out=ps[:], lhsT=lhsT(blk),
                        rhs=xt_g[:, blk * B:(blk + 1) * B],
                        start=(blk == 0), stop=(blk == KT - 1))
                rt = epool.tile([128, B], f32, tag="rt")
                nc.scalar.activation(
                    out=rt[:], in_=ps[:],
                    func=mybir.ActivationFunctionType.Relu,
                    scale=float(1.0 - ALPHA),
                    bias=cst_t[:, u:u + 1])
                ot = opool.tile([128, B], dt_x, tag="ot")
                nc.vector.scalar_tensor_tensor(
                    out=ot[:], in0=ps[:], scalar=ALPHA, in1=rt[:],
                    op0=mybir.AluOpType.mult, op1=mybir.AluOpType.add)
                nc.vector.tensor_scalar(
                    out=ot[:], in0=ot[:],
                    scalar1=cst_t[:, 2 * NU + u:2 * NU + u + 1],
                    scalar2=cst_t[:, NU + u:NU + u + 1],
                    op0=mybir.AluOpType.mult,
                    op1=mybir.AluOpType.add)
                nc.sync.dma_start(
                    out=out[u * 128:(u + 1) * 128, :], in_=ot[:])
    nc.compile()
    return nc


def _get_program(use_bf16: bool):
    key = (use_bf16, REPEAT)
    if key not in _prog_cache:
        _prog_cache[key] = _build_program(use_bf16)
    return _prog_cache[key]


def _pack(rows):
    """[NK*128, C] k-major rows -> [128, NK*C] tile layout (p, blk, c)."""
    nk = rows.shape[0] // 128
    c = rows.shape[1]
    return np.ascontiguousarray(
        rows.reshape(nk, 128, c).transpose(1, 0, 2)).reshape(128, nk * c)


def _prep_inputs(x, gidx, W, b, gamma, beta, mmean, mvar):
    dtx = _np_dtx()
    xT = np.ascontiguousarray(x.T)  # [D_IN, B]
    inv = (gamma.astype(np.float64)
           / np.sqrt(mvar.astype(np.float64) + BN_EPS))
    caff = beta.astype(np.float64) - mmean.astype(np.float64) * inv

    # full groups: core c -> [5c, 5c+5); group 40 split across cores 0/1
    in_maps, metas = [], []
    for c in range(N_CORES):
        gs = list(range(NF * c, NF * (c + 1)))
        if c < 2:
            hg, hh = N_GROUPS - 1, c      # real half unit
            half_real = True
        else:
            hg, hh = gs[-1], 0            # padded duplicate
            half_real = False
        xgroups = gs + [hg]
        # units: u<10 -> (gs[u//2], u%2); u=10 -> (hg, hh)
        units = [(gs[u // 2], u % 2) for u in range(2 * NF)] + [(hg, hh)]

        flat = gidx[xgroups].reshape(-1)      # [6*G]
        xg_p = _pack(xT[flat].astype(dtx))    # [128, XCOLS]
        wt_p = _pack(W[gs].reshape(NF * G, D_OUT).astype(dtx))
        wh_p = _pack(W[hg][:, hh * 128:(hh + 1) * 128].astype(dtx))

        cst = np.empty((128, NCOL), np.float32)
        for u, (gr, h) in enumerate(units):
            bs = b[gr, h * 128:(h + 1) * 128].astype(np.float64)
            iv = inv[h * 128:(h + 1) * 128]
            cf = caff[h * 128:(h + 1) * 128]
            cst[:, u] = (1.0 - ALPHA) * bs
            cst[:, NU + u] = cf + ALPHA * bs * iv
            cst[:, 2 * NU + u] = iv
        in_maps.append({"xg": xg_p, "wt": wt_p, "wh": wh_p,
                        "cst": np.ascontiguousarray(cst)})
        metas.append((units, half_real))
    return in_maps, metas


def _unshard(metas, get_out):
    full = np.empty((B, N_GROUPS, D_OUT), dtype=np.float32)
    for c, (units, half_real) in enumerate(metas):
        o = get_out(c).astype(np.float32).reshape(NU, 128, B)
        for u, (gr, h) in enumerate(units):
            if u == NU - 1 and not half_real:
                continue
            full[:, gr, h * 128:(h + 1) * 128] = o[u].T
    return full


def kernel(**inputs):
    x = np.asarray(inputs["x"], dtype=np.float32)
    gidx = np.asarray(inputs["group_idx"]).astype(np.int64)
    W = np.asarray(inputs["W"], dtype=np.float32)
    b = np.asarray(inputs["b"], dtype=np.float32)
    gamma = np.asarray(inputs["gamma"], dtype=np.float32)
    beta = np.asarray(inputs["beta"], dtype=np.float32)
    mmean = np.asarray(inputs["moving_mean"], dtype=np.float32)
    mvar = np.asarray(inputs["moving_var"], dtype=np.float32)

    in_maps, metas = _prep_inputs(x, gidx, W, b, gamma, beta, mmean, mvar)
    nc = _get_program(USE_BF16)

    from concourse import bass_utils
    res = bass_utils.run_bass_kernel_spmd(
        nc, in_maps, core_ids=list(range(N_CORES)), trace=TRACE, **TRACE_KW)
    if TRACE:
        kernel.last_result = res

    return _unshard(metas, lambda c: res.results[c]["out"])


def run_sim(cores=(0, 2)):
    """CoreSim validation of per-core programs (no hardware)."""
    import sys
    sys.path.insert(0, "/root/problem")
    from test import load_ref
    from concourse.bass_interp import CoreSim
    inputs, expected = load_ref()
    x = inputs["x"].astype(np.float32)
    gidx = inputs["group_idx"].astype(np.int64)
    in_maps, metas = _prep_inputs(
        x, gidx, inputs["W"].astype(np.float32), inputs["b"].astype(np.float32),
        inputs["gamma"].astype(np.float32), inputs["beta"].astype(np.float32),
        inputs["moving_mean"].astype(np.float32),
        inputs["moving_var"].astype(np.float32))
    nc = _get_program(USE_BF16)
    for core in cores:
        sim = CoreSim(nc)
        sim.assign_tensors(in_maps[core])
        sim.simulate(check_with_hw=False)
        o = sim.tensor("out").astype(np.float32).reshape(NU, 128, B)
        units, half_real = metas[core]
        errs = []
        for u, (gr, h) in enumerate(units):
            if u == NU - 1 and not half_real:
                continue
            exp = expected[:, gr, h * 128:(h + 1) * 128]
            errs.append(np.max(np.abs(o[u].T - exp)))
        err = max(errs) / (np.max(np.abs(expected)) + 1e-30)
        print(f"core {core}: sim max-abs-rel err = {err:.3e}")
    return err


if __name__ == "__main__":
    run_sim()
